# revision 9
# baseline (speedup 1.0000x reference)
"""MoChA stable chunkwise attention (window w=16) on 8 Trainium2 NeuronCores.

The reference's stabilizing moving-max cancels algebraically:
    P[t] = exp(logits[t]);  S[u] = sum_{v=u-15..u} P[v]
    R[u] = emit[u]/S[u];    out[t] = P[t] * sum_{k=0..15} R[t+k]
Both width-16 window sums run on the TensorEngine as banded matmuls in a
transposed layout: partition p = t mod 128, free column f = 8*(t//128) + row.
With that ordering the "previous block" of any column is exactly 8 columns
to the left, so the cross-block window wrap is two full-width matmuls against
shifted views of the same SBUF buffer (an 8-column zero pad supplies the
sequence-edge padding) — no masked-copy corner operands. Band weights are
fp8e5m2 (0/1 entries, exact); logits/emit travel fp16; output returns fp16
and is upcast on the host.

Scheduling: separate PSUM tiles per pipeline half keep dependencies
range-precise; the first Z-corner matmul is split 504/8 (its last 8 columns
need second-half R, which is produced early by a tiny 8-column rcp/mul
slice); exp runs in 256-column quarters chasing the logits DMAs; loads are
spread over the three DMA-capable rings (sync/scalar/gpsimd) ordered by
need-time; outputs go out on the two hardware rings only, so the slow
software-DGE drain happens early and off the exit path.

Self-contained: only numpy + ml_dtypes + concourse (on PYTHONPATH) required.
"""

import numpy as np
import ml_dtypes

import concourse.bass as bass
import concourse.tile as tile
import concourse.mybir as mybir
from concourse import bacc
from concourse.bass_utils import run_bass_kernel_spmd

F32 = mybir.dt.float32
F16 = mybir.dt.float16
F8E5 = mybir.dt.float8e5
ACTF = mybir.ActivationFunctionType

B, T = 64, 16384
NCORES = 8
RPC = B // NCORES        # 8 rows/core
NPART = 128
NBG = T // NPART         # 128 blocks of 128 t's per row
NF = RPC * NBG           # 1024 free columns
W = 16                   # window
PAD = RPC                # one block-shift = 8 columns

H0 = slice(0, 512)
H1 = slice(512, 1024)


def make_consts():
    k = np.arange(128)[:, None]
    m = np.arange(128)[None, :]
    band0 = (m - k >= 0) & (m - k <= W - 1)            # S within-block
    corner = (k - m >= 128 - W + 1) & (k - m <= 127)   # S from prev block
    banda = (k - m >= 0) & (k - m <= W - 1)            # Z within-block
    cornera = (m - k >= 128 - W + 1) & (m - k <= 127)  # Z from next block
    return np.concatenate(
        [x.astype(ml_dtypes.float8_e5m2)
         for x in (band0, corner, banda, cornera)],
        axis=1,
    )  # [128, 512] fp8e5m2


def _perm(a):
    """[RPC, T] -> device layout [128, NF]: f = 8*(t//128) + row."""
    return np.ascontiguousarray(
        a.reshape(RPC, NBG, NPART).transpose(2, 1, 0).reshape(NPART, NF)
    )


def unperm_out(o):
    """[128, NF] device layout -> [RPC, T]."""
    return np.ascontiguousarray(
        o.reshape(NPART, NBG, RPC).transpose(2, 1, 0).reshape(RPC, T)
    )


def build_nc():
    nc = bacc.Bacc("TRN2", target_bir_lowering=False, debug=False,
                   num_devices=NCORES)
    lg_t = nc.dram_tensor("lg16", [NPART, NF], F16, kind="ExternalInput")
    # emit packed two partitions per dram row -> 4KB lines on the wire
    em_t = nc.dram_tensor("em16", [NPART // 2, 2 * NF], F16,
                          kind="ExternalInput")
    kc_t = nc.dram_tensor("consts8", [NPART, 512], F8E5, kind="ExternalInput")
    out_t = nc.dram_tensor("out16", [NPART, NF], F16, kind="ExternalOutput")

    with tile.TileContext(nc) as tc:
        with (
            tc.tile_pool(name="sb", bufs=1) as sb,
            tc.tile_pool(name="ps", bufs=1, space="PSUM") as ps,
        ):
            kb = sb.tile([NPART, 512], F8E5, tag="kb")
            lg_b = sb.tile([NPART, NF], F16, tag="lg_b")
            e_b = sb.tile([NPART, NF], F16, tag="e_b")
            p_full = sb.tile([NPART, PAD + NF], F16, tag="p_full")
            rcp_b = sb.tile([NPART, NF], F32, tag="rcp_b")
            r_full = sb.tile([NPART, NF + PAD], F16, tag="r_full")
            o_b = sb.tile([NPART, NF], F16, tag="o_b")
            s_psA = ps.tile([NPART, 512], F32, tag="sA")
            s_psB = ps.tile([NPART, 512], F32, tag="sB")
            z_psA = ps.tile([NPART, 512], F32, tag="zA")
            z_psB = ps.tile([NPART, 512], F32, tag="zB")

            band0 = kb[:, 0:128]
            corner = kb[:, 128:256]
            banda = kb[:, 256:384]
            cornera = kb[:, 384:512]

            # P region of p_full is [PAD : PAD+NF]; col PAD+f holds P[f].
            pP = p_full[:, PAD:PAD + NF]

            # ---- loads over the three DMA rings, ordered by need-time.
            # 1KB+ lines everywhere; back-to-back transfers pipeline on a
            # ring at ~160GB/s ----
            # sync ring: both logits halves
            nc.sync.dma_start(lg_b[:, H0],
                              bass.AP(lg_t, 0, [[NF, NPART], [1, 512]]))
            # scalar ring (carries the exp ACT-table load first): S-weights
            nc.scalar.dma_start(kb[:, 0:256],
                                bass.AP(kc_t, 0, [[512, NPART], [1, 256]]))
            # gpsimd (software DGE): emit (packed 4KB lines), then Z-weights
            nc.gpsimd.dma_start(e_b[:, :],
                                bass.AP(em_t, 0, [[2 * NF, NPART // 2],
                                                  [1, 2 * NF]]))
            nc.sync.dma_start(lg_b[:, H1],
                              bass.AP(lg_t, 512, [[NF, NPART], [1, 512]]))
            nc.gpsimd.dma_start(kb[:, 256:512],
                                bass.AP(kc_t, 256, [[512, NPART], [1, 256]]))

            # zero pads: left pad of p_full, right pad of r_full
            nc.vector.memset(p_full[:, 0:PAD], 0.0)
            nc.vector.memset(r_full[:, NF:NF + PAD], 0.0)

            # ---- P = exp(logits), fp16, halves ----
            nc.scalar.activation(pP[:, H0], lg_b[:, H0], ACTF.Exp)
            nc.scalar.activation(pP[:, H1], lg_b[:, H1], ACTF.Exp)

            # ---- S = band0.T @ P + corner.T @ P(shifted one block left) ----
            nc.tensor.matmul(s_psA[:, :], band0, pP[:, H0],
                             start=True, stop=False, skip_group_check=True)
            nc.tensor.matmul(s_psA[:, :], corner, p_full[:, 0:512],
                             start=False, stop=True, skip_group_check=True)
            nc.tensor.matmul(s_psB[:, :], band0, pP[:, H1],
                             start=True, stop=False, skip_group_check=True)
            nc.tensor.matmul(s_psB[:, :], corner, p_full[:, 512:1024],
                             start=False, stop=True, skip_group_check=True)

            # ---- R = emit / S.  The first 8 columns of the second half are
            # produced early: they unblock the first half's Z-corner tail ----
            nc.vector.reciprocal_approx_fast(rcp_b[:, H0], s_psA[:, :])
            nc.vector.tensor_mul(r_full[:, H0], e_b[:, H0], rcp_b[:, H0])
            nc.vector.reciprocal_approx_fast(rcp_b[:, 512:520], s_psB[:, 0:8])
            nc.vector.tensor_mul(r_full[:, 512:520], e_b[:, 512:520],
                                 rcp_b[:, 512:520])
            nc.vector.reciprocal_approx_fast(rcp_b[:, 520:1024],
                                             s_psB[:, 8:512])
            nc.vector.tensor_mul(r_full[:, 520:1024], e_b[:, 520:1024],
                                 rcp_b[:, 520:1024])

            # ---- Z = banda.T @ R + cornera.T @ R(shifted one block right).
            # First-half corner split 504/8 ----
            nc.tensor.matmul(z_psA[:, :], banda, r_full[:, H0],
                             start=True, stop=False, skip_group_check=True)
            nc.tensor.matmul(z_psA[:, 0:504], cornera, r_full[:, PAD:512],
                             start=False, stop=False, skip_group_check=True)
            nc.tensor.matmul(z_psA[:, 504:512], cornera, r_full[:, 512:520],
                             start=False, stop=True, skip_group_check=True)
            nc.tensor.matmul(z_psB[:, :], banda, r_full[:, H1],
                             start=True, stop=False, skip_group_check=True)
            nc.tensor.matmul(z_psB[:, :], cornera, r_full[:, PAD + 512:PAD + NF],
                             start=False, stop=True, skip_group_check=True)

            # ---- out = P * Z (fp16); partition-sliced 1KB-line pieces
            # criss-crossed over the two HW rings ----
            nc.vector.tensor_mul(o_b[:, H0], pP[:, H0], z_psA[:, :])
            nc.sync.dma_start(bass.AP(out_t, 0, [[NF, 64], [1, 512]]),
                              o_b[0:64, H0])
            nc.scalar.dma_start(bass.AP(out_t, 64 * NF, [[NF, 64], [1, 512]]),
                                o_b[64:128, H0])
            nc.vector.tensor_mul(o_b[:, H1], pP[:, H1], z_psB[:, :])
            nc.sync.dma_start(bass.AP(out_t, 512, [[NF, 64], [1, 512]]),
                              o_b[0:64, H1])
            nc.scalar.dma_start(
                bass.AP(out_t, 64 * NF + 512, [[NF, 64], [1, 512]]),
                o_b[64:128, H1])

    nc.compile()
    return nc


def make_in_maps(emit_probs, softmax_logits):
    lg16 = np.asarray(softmax_logits, dtype=np.float16)
    em16 = np.asarray(emit_probs, dtype=np.float16)
    consts = make_consts()
    maps = []
    for k in range(NCORES):
        rows = slice(k * RPC, (k + 1) * RPC)
        maps.append({
            "lg16": _perm(lg16[rows]),
            "em16": _perm(em16[rows]).reshape(NPART // 2, 2 * NF),
            "consts8": consts,
        })
    return maps


_NC_CACHE = None


def _get_nc():
    global _NC_CACHE
    if _NC_CACHE is None:
        _NC_CACHE = build_nc()
    return _NC_CACHE


def run(emit_probs, softmax_logits, trace=False, **kwargs):
    nc = _get_nc()
    in_maps = make_in_maps(emit_probs, softmax_logits)
    res = run_bass_kernel_spmd(
        nc, in_maps, core_ids=list(range(NCORES)), trace=trace, **kwargs
    )
    out = np.concatenate(
        [unperm_out(res.results[k]["out16"]).astype(np.float32)
         for k in range(NCORES)],
        axis=0,
    )
    return out, res


def kernel(emit_probs, softmax_logits):
    return run(emit_probs, softmax_logits)[0]


# revision 11
# speedup vs baseline: 1.0400x; 1.0400x over previous
"""MoChA stable chunkwise attention (window w=16) on 8 Trainium2 NeuronCores.

The reference's stabilizing moving-max cancels algebraically:
    P[t] = exp(logits[t]);  S[u] = sum_{v=u-15..u} P[v]
    R[u] = emit[u]/S[u];    out[t] = P[t] * sum_{k=0..15} R[t+k]
Both width-16 window sums run on the TensorEngine as banded matmuls in a
transposed layout: partition p = t mod 128, free column f = 8*(t//128) + row.
With that ordering the "previous block" of any column is exactly 8 columns
to the left, so the cross-block window wrap is two full-width matmuls against
shifted views of the same SBUF buffer (an 8-column zero pad supplies the
sequence-edge padding) — no masked-copy corner operands.

The four banded weight matrices are generated on-device (memset +
affine_select on the otherwise idle GpSimd engine) so no ring bandwidth is
spent on them. Logits halves load on the two HW rings, emit (packed two
partitions per dram row for 4KB lines) on the software-DGE ring. Separate
PSUM tiles per pipeline stage keep dependencies range-precise; the second
half's divide/window/store path runs in 8/256/248-column slices so its
output DMAs start as early as possible. Output returns fp16, upcast on host.

Self-contained: only numpy + concourse (on PYTHONPATH) required.
"""

import numpy as np

import concourse.bass as bass
import concourse.tile as tile
import concourse.mybir as mybir
from concourse import bacc
from concourse.bass_utils import run_bass_kernel_spmd

F32 = mybir.dt.float32
F16 = mybir.dt.float16
ACTF = mybir.ActivationFunctionType
ALU = mybir.AluOpType

B, T = 64, 16384
NCORES = 8
RPC = B // NCORES        # 8 rows/core
NPART = 128
NBG = T // NPART         # 128 blocks of 128 t's per row
NF = RPC * NBG           # 1024 free columns
W = 16                   # window
PAD = RPC                # one block-shift = 8 columns

H0 = slice(0, 512)
H1 = slice(512, 1024)


def _perm(a):
    """[RPC, T] -> device layout [128, NF]: f = 8*(t//128) + row."""
    return np.ascontiguousarray(
        a.reshape(RPC, NBG, NPART).transpose(2, 1, 0).reshape(NPART, NF)
    )


def unperm_out(o):
    """[128, NF] device layout -> [RPC, T]."""
    return np.ascontiguousarray(
        o.reshape(NPART, NBG, RPC).transpose(2, 1, 0).reshape(RPC, T)
    )


def _gen_consts(nc, kb):
    """Band weights, built in-place on GpSimd: band0[k,m]=1 iff 0<=m-k<=15;
    corner[k,m]=1 iff k-m in [113,127]; banda/cornera are the transposes."""
    g = nc.gpsimd
    g.memset(kb[:, :], 1.0)
    # only is_ge lowers in walrus codegen; every band test is phrased >= 0
    # band0: keep (m-k >= 0) and (15-m+k >= 0)
    g.affine_select(kb[:, 0:128], kb[:, 0:128], [[1, 128]], ALU.is_ge, 0.0,
                    base=0, channel_multiplier=-1)
    g.affine_select(kb[:, 0:128], kb[:, 0:128], [[-1, 128]], ALU.is_ge, 0.0,
                    base=W - 1, channel_multiplier=1)
    # corner: keep (k-m-113 >= 0)
    g.affine_select(kb[:, 128:256], kb[:, 128:256], [[-1, 128]], ALU.is_ge,
                    0.0, base=-(128 - W + 1), channel_multiplier=1)
    # banda: keep (k-m >= 0) and (15-k+m >= 0)
    g.affine_select(kb[:, 256:384], kb[:, 256:384], [[-1, 128]], ALU.is_ge,
                    0.0, base=0, channel_multiplier=1)
    g.affine_select(kb[:, 256:384], kb[:, 256:384], [[1, 128]], ALU.is_ge,
                    0.0, base=W - 1, channel_multiplier=-1)
    # cornera: keep (m-k-113 >= 0)
    g.affine_select(kb[:, 384:512], kb[:, 384:512], [[1, 128]], ALU.is_ge,
                    0.0, base=-(128 - W + 1), channel_multiplier=-1)


def build_nc():
    nc = bacc.Bacc("TRN2", target_bir_lowering=False, debug=False,
                   num_devices=NCORES)
    lg_t = nc.dram_tensor("lg16", [NPART, NF], F16, kind="ExternalInput")
    # emit packed two partitions per dram row -> 4KB lines on the wire
    em_t = nc.dram_tensor("em16", [NPART // 2, 2 * NF], F16,
                          kind="ExternalInput")
    out_t = nc.dram_tensor("out16", [NPART, NF], F16, kind="ExternalOutput")

    with tile.TileContext(nc) as tc:
        with (
            tc.tile_pool(name="sb", bufs=1) as sb,
            tc.tile_pool(name="ps", bufs=1, space="PSUM") as ps,
        ):
            kb = sb.tile([NPART, 512], F16, tag="kb")
            lg_b = sb.tile([NPART, NF], F16, tag="lg_b")
            e_b = sb.tile([NPART, NF], F16, tag="e_b")
            p_full = sb.tile([NPART, PAD + NF], F16, tag="p_full")
            rcp_b = sb.tile([NPART, NF], F32, tag="rcp_b")
            r_full = sb.tile([NPART, NF + PAD], F16, tag="r_full")
            o_b = sb.tile([NPART, NF], F16, tag="o_b")
            s_psA = ps.tile([NPART, 512], F32, tag="sA")
            s_psB = ps.tile([NPART, 512], F32, tag="sB")
            z_psA = ps.tile([NPART, 512], F32, tag="zA")
            z_psB = ps.tile([NPART, 512], F32, tag="zB")

            band0 = kb[:, 0:128]
            corner = kb[:, 128:256]
            banda = kb[:, 256:384]
            cornera = kb[:, 384:512]

            # P region of p_full is [PAD : PAD+NF]; col PAD+f holds P[f].
            pP = p_full[:, PAD:PAD + NF]

            # ---- loads: one logits half per HW ring, emit on the software
            # ring; weights are generated, not loaded ----
            nc.sync.dma_start(lg_b[:, H0],
                              bass.AP(lg_t, 0, [[NF, NPART], [1, 512]]))
            nc.scalar.dma_start(lg_b[:, H1],
                                bass.AP(lg_t, 512, [[NF, NPART], [1, 512]]))
            nc.gpsimd.dma_start(e_b[:, :],
                                bass.AP(em_t, 0, [[2 * NF, NPART // 2],
                                                  [1, 2 * NF]]))
            _gen_consts(nc, kb)

            # zero pads: left pad of p_full, right pad of r_full
            nc.vector.memset(p_full[:, 0:PAD], 0.0)
            nc.vector.memset(r_full[:, NF:NF + PAD], 0.0)

            # ---- P = exp(logits), fp16, halves ----
            nc.scalar.activation(pP[:, H0], lg_b[:, H0], ACTF.Exp)
            nc.scalar.activation(pP[:, H1], lg_b[:, H1], ACTF.Exp)

            # ---- S = band0.T @ P + corner.T @ P(shifted one block left) ----
            nc.tensor.matmul(s_psA[:, :], band0, pP[:, H0],
                             start=True, stop=False, skip_group_check=True)
            nc.tensor.matmul(s_psA[:, :], corner, p_full[:, 0:512],
                             start=False, stop=True, skip_group_check=True)
            nc.tensor.matmul(s_psB[:, :], band0, pP[:, H1],
                             start=True, stop=False, skip_group_check=True)
            nc.tensor.matmul(s_psB[:, :], corner, p_full[:, 512:1024],
                             start=False, stop=True, skip_group_check=True)

            # ---- R = emit / S.  Half B in 8/256/248-column slices so
            # downstream work starts as early as possible ----
            nc.vector.reciprocal_approx_fast(rcp_b[:, H0], s_psA[:, :])
            nc.vector.tensor_mul(r_full[:, H0], e_b[:, H0], rcp_b[:, H0])
            nc.vector.reciprocal_approx_fast(rcp_b[:, 512:520], s_psB[:, 0:8])
            nc.vector.tensor_mul(r_full[:, 512:520], e_b[:, 512:520],
                                 rcp_b[:, 512:520])
            nc.vector.reciprocal_approx_fast(rcp_b[:, 520:776],
                                             s_psB[:, 8:264])
            nc.vector.tensor_mul(r_full[:, 520:776], e_b[:, 520:776],
                                 rcp_b[:, 520:776])
            nc.vector.reciprocal_approx_fast(rcp_b[:, 776:1024],
                                             s_psB[:, 264:512])
            nc.vector.tensor_mul(r_full[:, 776:1024], e_b[:, 776:1024],
                                 rcp_b[:, 776:1024])

            # ---- Z = banda.T @ R + cornera.T @ R(shifted one block right).
            # Half A's corner is split 504/8 (tail reads early half-B R);
            # half B runs in two 256-column pieces ----
            nc.tensor.matmul(z_psA[:, :], banda, r_full[:, H0],
                             start=True, stop=False, skip_group_check=True)
            nc.tensor.matmul(z_psA[:, 0:504], cornera, r_full[:, PAD:512],
                             start=False, stop=False, skip_group_check=True)
            nc.tensor.matmul(z_psA[:, 504:512], cornera, r_full[:, 512:520],
                             start=False, stop=True, skip_group_check=True)
            nc.tensor.matmul(z_psB[:, 0:256], banda, r_full[:, 512:768],
                             start=True, stop=False, skip_group_check=True)
            nc.tensor.matmul(z_psB[:, 0:256], cornera, r_full[:, 520:776],
                             start=False, stop=True, skip_group_check=True)
            nc.tensor.matmul(z_psB[:, 256:512], banda, r_full[:, 768:1024],
                             start=True, stop=False, skip_group_check=True)
            nc.tensor.matmul(z_psB[:, 256:512], cornera,
                             r_full[:, 776:1032],
                             start=False, stop=True, skip_group_check=True)

            # ---- out = P * Z (fp16) in 256-column pieces, DMA'd on the two
            # HW rings as soon as each piece lands ----
            nc.vector.tensor_mul(o_b[:, 0:256], pP[:, 0:256], z_psA[:, 0:256])
            nc.sync.dma_start(bass.AP(out_t, 0, [[NF, NPART], [1, 256]]),
                              o_b[:, 0:256])
            nc.vector.tensor_mul(o_b[:, 256:512], pP[:, 256:512],
                                 z_psA[:, 256:512])
            nc.scalar.dma_start(bass.AP(out_t, 256, [[NF, NPART], [1, 256]]),
                                o_b[:, 256:512])
            nc.vector.tensor_mul(o_b[:, 512:768], pP[:, 512:768],
                                 z_psB[:, 0:256])
            nc.sync.dma_start(bass.AP(out_t, 512, [[NF, NPART], [1, 256]]),
                              o_b[:, 512:768])
            nc.vector.tensor_mul(o_b[:, 768:1024], pP[:, 768:1024],
                                 z_psB[:, 256:512])
            nc.scalar.dma_start(bass.AP(out_t, 768, [[NF, NPART], [1, 256]]),
                                o_b[:, 768:1024])

    nc.compile()
    return nc


def make_in_maps(emit_probs, softmax_logits):
    lg16 = np.asarray(softmax_logits, dtype=np.float16)
    em16 = np.asarray(emit_probs, dtype=np.float16)
    maps = []
    for k in range(NCORES):
        rows = slice(k * RPC, (k + 1) * RPC)
        maps.append({
            "lg16": _perm(lg16[rows]),
            "em16": _perm(em16[rows]).reshape(NPART // 2, 2 * NF),
        })
    return maps


_NC_CACHE = None


def _get_nc():
    global _NC_CACHE
    if _NC_CACHE is None:
        _NC_CACHE = build_nc()
    return _NC_CACHE


def run(emit_probs, softmax_logits, trace=False, **kwargs):
    nc = _get_nc()
    in_maps = make_in_maps(emit_probs, softmax_logits)
    res = run_bass_kernel_spmd(
        nc, in_maps, core_ids=list(range(NCORES)), trace=trace, **kwargs
    )
    out = np.concatenate(
        [unperm_out(res.results[k]["out16"]).astype(np.float32)
         for k in range(NCORES)],
        axis=0,
    )
    return out, res


def kernel(emit_probs, softmax_logits):
    return run(emit_probs, softmax_logits)[0]
